# revision 13
# baseline (speedup 1.0000x reference)
"""AutoResetRNN (masked GRU) TRN2 kernel.

Data-parallel over N across 8 NeuronCores (N_loc = 32 envs/core).
Per core:
  - phase A: gx = x @ w_ih.T + b_comb as a blocked f32r GEMM, stored bf16 in SBUF
  - recurrence: per-step col-tiled f32r GEMM waves accumulating the three
    gate pre-activations in PSUM, elementwise tail on DVE/ACT in a
    "half-major" (64,256) layout (partition p = e + 32*half, free = H%256),
    PE transpose (with mask folded via the next step's ts) to produce hT.

All host-side work here is layout only (shard/transpose/reshape); all math
runs on the NeuronCores.
"""

import numpy as np

import concourse.bass as bass
import concourse.bacc as bacc
import concourse.mybir as mybir
import concourse.tile as tile
from concourse import bass_utils

F32 = mybir.dt.float32
F32R = mybir.dt.float32r
BF16 = mybir.dt.bfloat16
AF = mybir.ActivationFunctionType
OP = mybir.AluOpType

T, N, D, H = 512, 256, 512, 512
NCORE = 8
NLOC = N // NCORE          # 32 envs per core
TB = 32                    # time-block (recurrence steps per phase-A block)
NBLK = T // TB
GX_PER_BLK = TB // 4       # gx tiles (128 rows = 4 steps x 32 envs) per block

# hT chunk k lives at free columns CHUNK_COLS[k] of the (128,128) hT tile
# (k = 2*half + c where c is the transpose block; see transpose emit below).
CHUNK_COLS = {0: 0, 1: 64, 2: 32, 3: 96}


def r32(ap):
    return ap.bitcast(F32R)


def build_nc(t_steps=T):
    nblk = t_steps // TB
    nc = bacc.Bacc("TRN2", target_bir_lowering=False, debug=False)

    xt = nc.dram_tensor("xt", [t_steps // 4, 128, 512], F32R, kind="ExternalInput")
    wih_t = nc.dram_tensor("wih_t", [512, 1536], F32R, kind="ExternalInput")
    whh_t = nc.dram_tensor("whh_t", [512, 1536], BF16, kind="ExternalInput")
    bcomb = nc.dram_tensor("bcomb", [1, 1536], F32R, kind="ExternalInput")
    bhn = nc.dram_tensor("bhn", [1, 512], BF16, kind="ExternalInput")
    m2d = nc.dram_tensor("m2", [64, t_steps], F32, kind="ExternalInput")
    h0d = nc.dram_tensor("h0", [32, 512], F32, kind="ExternalInput")
    ysd = nc.dram_tensor("ys", [t_steps, 32, 512], F32, kind="ExternalOutput")

    with tile.TileContext(nc) as tc:
        with (
            tc.tile_pool(name="const", bufs=1) as cpool,
            tc.tile_pool(name="wpool", bufs=1) as wpool,
            tc.tile_pool(name="gx", bufs=2 * GX_PER_BLK) as gxpool,
            tc.tile_pool(name="xstage", bufs=3) as xpool,
            tc.tile_pool(name="ev", bufs=2) as evpool,
            tc.tile_pool(name="ht", bufs=2) as htpool,
            tc.tile_pool(name="psA", bufs=2, space="PSUM") as psA,
            tc.tile_pool(name="psB", bufs=2, space="PSUM") as psB,
            tc.tile_pool(name="psT", bufs=2, space="PSUM") as psT,
            tc.tile_pool(name="psG", bufs=2, space="PSUM") as psG,
        ):
            # ---- constants / weights ----
            import ml_dtypes
            id128_d = nc.inline_tensor(
                np.eye(128).astype(ml_dtypes.bfloat16), name="id128c")
            id64_d = nc.inline_tensor(np.eye(64, dtype=np.float32), name="id64c")
            ones_d = nc.inline_tensor(np.ones((1, 128), np.float32), name="onesc")
            ident128b = cpool.tile([128, 128], BF16, tag="id128")
            nc.sync.dma_start(ident128b[:, :], id128_d.ap())
            ident64 = cpool.tile([64, 64], F32, tag="id64")
            nc.sync.dma_start(ident64[:, :], id64_d.ap())
            ones1 = cpool.tile([1, 128], F32R, tag="ones")
            nc.sync.dma_start(ones1[:, :], ones_d.ap().bitcast(F32R))
            onesb_d = nc.inline_tensor(
                np.ones((1, 128)).astype(ml_dtypes.bfloat16), name="onesbc")
            ones1b = cpool.tile([1, 128], BF16, tag="onesb")
            nc.sync.dma_start(ones1b[:, :], onesb_d.ap())

            wih = [wpool.tile([128, 1536], F32R, tag=f"wih{k}", name=f"wih{k}") for k in range(4)]
            whh = [wpool.tile([128, 1536], BF16, tag=f"whh{k}", name=f"whh{k}") for k in range(4)]
            for k in range(4):
                nc.sync.dma_start(wih[k][:, :], wih_t[128 * k:128 * (k + 1), :])
                nc.sync.dma_start(whh[k][:, :], whh_t[128 * k:128 * (k + 1), :])
            bcomb_sb = cpool.tile([1, 1536], F32R, tag="bcomb")
            nc.sync.dma_start(bcomb_sb[:, :], bcomb[:, :])
            bhn_sb = cpool.tile([1, 512], BF16, tag="bhn")
            nc.sync.dma_start(bhn_sb[:, :], bhn[:, :])
            m2sb = cpool.tile([64, t_steps], F32, tag="m2")
            nc.sync.dma_start(m2sb[:, :], m2d[:, :])

            # ---- phase A emitters ----
            gx_tiles = {}

            def phase_a_dma(g):
                xs = xpool.tile([128, 512], F32R, tag="xstage", name="xs")
                nc.sync.dma_start(xs[:, :], xt[g, :, :])
                gxt = gxpool.tile([128, 1536], BF16, tag="gx", name="gxt")
                gx_tiles[g] = (gxt, xs)
                return gxt, xs

            def phase_a_chunk_mm(g, nchunk):
                gxt, xs = gx_tiles[g]
                gp = psG.tile([128, 512], F32, tag="gxp", name="gp")
                nc.tensor.matmul(
                    gp[:, :], r32(ones1[:, :]),
                    r32(bcomb_sb[:, 512 * nchunk:512 * (nchunk + 1)]),
                    start=True, stop=False)
                for k in range(4):
                    nc.tensor.matmul(
                        gp[:, :], r32(xs[:, 128 * k:128 * (k + 1)]),
                        r32(wih[k][:, 512 * nchunk:512 * (nchunk + 1)]),
                        start=False, stop=(k == 3))
                return gp

            def phase_a_chunk_copy(g, nchunk, gp):
                gxt, _ = gx_tiles[g]
                nc.vector.tensor_copy(
                    gxt[:, 512 * nchunk:512 * (nchunk + 1)], gp[:, :])

            def phase_a_chunk(g, nchunk):
                gp = phase_a_chunk_mm(g, nchunk)
                phase_a_chunk_copy(g, nchunk, gp)

            # ---- prologue: h0 -> half-major layout, initial masked transpose ----
            h_cur = evpool.tile([64, 256], F32, tag="h")
            nc.sync.dma_start(
                h_cur[:, :], h0d.ap().rearrange("e (h j) -> h e j", h=2))

            def emit_transpose(h_src, t_next):
                """hm = m[t_next]*h_src; hT = transpose(hm) into SBUF f32."""
                hm = evpool.tile([64, 256], F32, tag="hm", name="hm")
                nc.vector.tensor_scalar_mul(
                    hm[:, :], h_src[:, :], m2sb[:, t_next:t_next + 1])
                tp = psT.tile([128, 128], F32, tag="htp", name="tp")
                ht = htpool.tile([128, 128], BF16, tag="ht", name="ht")
                for c in range(2):
                    nc.tensor.transpose(
                        tp[:, 64 * c:64 * (c + 1)],
                        hm[:, 128 * c:128 * (c + 1)], ident64[:, :])
                    nc.scalar.activation(
                        ht[:, 64 * c:64 * (c + 1)],
                        tp[:, 64 * c:64 * (c + 1)], AF.Copy)
                return ht, hm

            ht_cur, hm_cur = emit_transpose(h_cur, 0)

            # phase A for block 0 (not interleaved)
            for g in range(GX_PER_BLK):
                phase_a_dma(g)
                for nchunk in range(3):
                    phase_a_chunk(g, nchunk)

            # ---- main loop ----
            for blk in range(nblk):
                for i in range(TB):
                    t = blk * TB + i
                    g = t // 4
                    b = 32 * (t % 4)
                    gxt = gx_tiles[g][0]

                    pa = psA.tile([128, 256], F32, tag="pa")
                    pb = psB.tile([128, 256], F32, tag="pb")

                    # W0a (bank B): bias(hn) + xn selector
                    nc.tensor.matmul(pb[0:32, :], ones1b[:, 0:32],
                                     bhn_sb[:, 0:256],
                                     start=True, stop=False,
                                     skip_group_check=True,
                                     tile_position=(0, 0))
                    nc.tensor.matmul(pb[32:64, :], ones1b[:, 0:32],
                                     bhn_sb[:, 256:512],
                                     start=True, stop=False,
                                     skip_group_check=True,
                                     tile_position=(0, 32))
                    nc.tensor.matmul(pb[64:96, :], ident128b[:, b:b + 32],
                                     gxt[:, 1024:1280],
                                     start=True, stop=True,
                                     skip_group_check=True,
                                     tile_position=(0, 64))
                    nc.tensor.matmul(pb[96:128, :], ident128b[:, b:b + 32],
                                     gxt[:, 1280:1536],
                                     start=True, stop=True,
                                     skip_group_check=True,
                                     tile_position=(0, 96))
                    # W0b (bank A): xz + xr selector
                    nc.tensor.matmul(pa[0:32, :], ident128b[:, b:b + 32],
                                     gxt[:, 512:768],
                                     start=True, stop=False,
                                     skip_group_check=True,
                                     tile_position=(0, 0))
                    nc.tensor.matmul(pa[32:64, :], ident128b[:, b:b + 32],
                                     gxt[:, 768:1024],
                                     start=True, stop=False,
                                     skip_group_check=True,
                                     tile_position=(0, 32))
                    nc.tensor.matmul(pa[64:96, :], ident128b[:, b:b + 32],
                                     gxt[:, 0:256],
                                     start=True, stop=False,
                                     skip_group_check=True,
                                     tile_position=(0, 64))
                    nc.tensor.matmul(pa[96:128, :], ident128b[:, b:b + 32],
                                     gxt[:, 256:512],
                                     start=True, stop=False,
                                     skip_group_check=True,
                                     tile_position=(0, 96))
                    # W1-4: hn (bank B groups 0,1) + r (bank A groups 2,3)
                    for k in (0, 2, 1, 3):
                        lh = ht_cur[:, CHUNK_COLS[k]:CHUNK_COLS[k] + 32]
                        nc.tensor.matmul(pb[0:32, :], lh,
                                         whh[k][:, 1024:1280],
                                         start=False, stop=(k == 3),
                                         skip_group_check=True,
                                         tile_position=(0, 0))
                        nc.tensor.matmul(pb[32:64, :], lh,
                                         whh[k][:, 1280:1536],
                                         start=False, stop=(k == 3),
                                         skip_group_check=True,
                                         tile_position=(0, 32))
                        nc.tensor.matmul(pa[64:96, :], lh,
                                         whh[k][:, 0:256],
                                         start=False, stop=(k == 3),
                                         skip_group_check=True,
                                         tile_position=(0, 64))
                        nc.tensor.matmul(pa[96:128, :], lh,
                                         whh[k][:, 256:512],
                                         start=False, stop=(k == 3),
                                         skip_group_check=True,
                                         tile_position=(0, 96))
                    # W5-8: z (bank A groups 0,1)
                    for k in (0, 2, 1, 3):
                        lh = ht_cur[:, CHUNK_COLS[k]:CHUNK_COLS[k] + 32]
                        nc.tensor.matmul(pa[0:32, :], lh,
                                         whh[k][:, 512:768],
                                         start=False, stop=(k == 3),
                                         skip_group_check=True,
                                         tile_position=(0, 0))
                        nc.tensor.matmul(pa[32:64, :], lh,
                                         whh[k][:, 768:1024],
                                         start=False, stop=(k == 3),
                                         skip_group_check=True,
                                         tile_position=(0, 32))

                    # interleaved phase A quantum for next block (MM part)
                    pa_copy = None
                    if blk + 1 < nblk:
                        gq = (blk + 1) * GX_PER_BLK + i // 4
                        quantum = i % 4
                        if quantum == 0:
                            phase_a_dma(gq)
                        else:
                            gp_q = phase_a_chunk_mm(gq, quantum - 1)
                            pa_copy = (gq, quantum - 1, gp_q)

                    # ---- elementwise tail ----
                    r_sb = evpool.tile([64, 256], F32, tag="r")
                    nc.scalar.activation(r_sb[:, :], pa[64:128, :], AF.Sigmoid)
                    z_sb = evpool.tile([64, 256], F32, tag="z")
                    nc.scalar.activation(z_sb[:, :], pa[0:64, :], AF.Sigmoid)
                    p_sb = evpool.tile([64, 256], F32, tag="p")
                    nc.vector.tensor_tensor(p_sb[:, :], r_sb[:, :],
                                            pb[0:64, :], OP.mult)
                    q_sb = evpool.tile([64, 256], F32, tag="q")
                    nc.vector.tensor_tensor(q_sb[:, :], p_sb[:, :],
                                            pb[64:128, :], OP.add)
                    n_sb = evpool.tile([64, 256], F32, tag="n")
                    nc.scalar.activation(n_sb[:, :], q_sb[:, :], AF.Tanh)
                    d_sb = evpool.tile([64, 256], F32, tag="d")
                    nc.vector.tensor_tensor(d_sb[:, :], hm_cur[:, :],
                                            n_sb[:, :], OP.subtract)
                    e_sb = evpool.tile([64, 256], F32, tag="e")
                    nc.vector.tensor_tensor(e_sb[:, :], d_sb[:, :],
                                            z_sb[:, :], OP.mult)
                    h_new = evpool.tile([64, 256], F32, tag="h")
                    nc.vector.tensor_tensor(h_new[:, :], e_sb[:, :],
                                            n_sb[:, :], OP.add)
                    nc.sync.dma_start(
                        ysd[t].rearrange("e (h j) -> h e j", h=2),
                        h_new[:, :])
                    if t + 1 < t_steps:
                        ht_cur, hm_cur = emit_transpose(h_new, t + 1)
                    if pa_copy is not None:
                        phase_a_chunk_copy(*pa_copy)
                    h_cur = h_new
    nc.compile()
    return nc


def _prep_core_inputs(x3, m3, h0, t_steps):
    """x3: (T, 32, 512), m3: (T, 32), h0: (32, 512) -> in_map dict."""
    # xt[g, dd, (k, tt, e)] = x3[4g + tt, e, 128k + dd]
    x4 = x3.reshape(t_steps // 4, 4, NLOC, 4, 128)        # g, tt, e, k, dd
    xt = np.ascontiguousarray(x4.transpose(0, 4, 3, 1, 2)  # g, dd, k, tt, e
                              ).reshape(t_steps // 4, 128, 512)
    m2 = np.ascontiguousarray(
        np.concatenate([m3.T, m3.T], axis=0))              # (64, T)
    return {"xt": xt, "m2": m2, "h0": np.ascontiguousarray(h0)}


def kernel(x, hxs, masks, w_ih, w_hh, b_ih, b_hh, _nc_cache={}):
    t_steps = T
    x = np.asarray(x, np.float32)
    hxs = np.asarray(hxs, np.float32)
    masks = np.asarray(masks, np.float32)
    w_ih = np.asarray(w_ih, np.float32)
    w_hh = np.asarray(w_hh, np.float32)
    b_ih = np.asarray(b_ih, np.float32)
    b_hh = np.asarray(b_hh, np.float32)

    import ml_dtypes
    wih_t = np.ascontiguousarray(w_ih.T)                   # (512, 1536)
    whh_t = np.ascontiguousarray(w_hh.T).astype(ml_dtypes.bfloat16)
    bc = b_ih.copy()
    bc[:1024] += b_hh[:1024]
    bcomb = bc[None, :]                                    # (1, 1536)
    bhn = b_hh[None, 1024:1536].astype(ml_dtypes.bfloat16)  # (1, 512)

    x3f = x.reshape(t_steps, N, D)
    m3f = masks.reshape(t_steps, N)
    h0f = hxs[:, 0, :]                                     # (N, H)

    shared = {"wih_t": wih_t, "whh_t": whh_t, "bcomb": bcomb, "bhn": bhn}
    in_maps = []
    for c in range(NCORE):
        sl = slice(NLOC * c, NLOC * (c + 1))
        im = _prep_core_inputs(
            np.ascontiguousarray(x3f[:, sl]), m3f[:, sl], h0f[sl], t_steps)
        im.update({k: v.copy() for k, v in shared.items()})
        in_maps.append(im)

    if "nc" not in _nc_cache:
        _nc_cache["nc"] = build_nc(t_steps)
    nc = _nc_cache["nc"]
    res = bass_utils.run_bass_kernel_spmd(nc, in_maps, core_ids=list(range(NCORE)))
    ys = np.concatenate(
        [r["ys"] for r in res.results], axis=1)            # (T, 256, 512)
    ys_flat = np.ascontiguousarray(ys).reshape(t_steps * N, H)
    h_last = np.ascontiguousarray(ys[-1])[None]            # (1, N, H)
    return ys_flat, h_last


# revision 16
# speedup vs baseline: 1.3701x; 1.3701x over previous
"""AutoResetRNN (masked GRU) TRN2 kernel.

Data-parallel over N across 8 NeuronCores (N_loc = 32 envs/core).
Per core:
  - phase A: gx = x @ w_ih.T + b_comb as a blocked f32r GEMM, stored bf16 in SBUF
  - recurrence: per-step col-tiled f32r GEMM waves accumulating the three
    gate pre-activations in PSUM, elementwise tail on DVE/ACT in a
    "half-major" (64,256) layout (partition p = e + 32*half, free = H%256),
    PE transpose (with mask folded via the next step's ts) to produce hT.

All host-side work here is layout only (shard/transpose/reshape); all math
runs on the NeuronCores.
"""

import numpy as np

import concourse.bass as bass
import concourse.bacc as bacc
import concourse.mybir as mybir
import concourse.tile as tile
from concourse import bass_utils

F32 = mybir.dt.float32
F32R = mybir.dt.float32r
BF16 = mybir.dt.bfloat16
AF = mybir.ActivationFunctionType
OP = mybir.AluOpType

T, N, D, H = 512, 256, 512, 512
NCORE = 8
NLOC = N // NCORE          # 32 envs per core
TB = 32                    # time-block (recurrence steps per phase-A block)
NBLK = T // TB
GX_PER_BLK = TB // 4       # gx tiles (128 rows = 4 steps x 32 envs) per block

# hT chunk k lives at free columns CHUNK_COLS[k] of the (128,128) hT tile
# (k = 2*half + c where c is the transpose block; see transpose emit below).
CHUNK_COLS = {0: 0, 1: 64, 2: 32, 3: 96}


def r32(ap):
    return ap.bitcast(F32R)


def build_nc(t_steps=T):
    nblk = t_steps // TB
    nc = bacc.Bacc("TRN2", target_bir_lowering=False, debug=False)

    xt = nc.dram_tensor("xt", [t_steps // 4, 128, 512], F32R, kind="ExternalInput")
    wih_t = nc.dram_tensor("wih_t", [512, 1536], F32R, kind="ExternalInput")
    whh_t = nc.dram_tensor("whh_t", [512, 1536], BF16, kind="ExternalInput")
    bcomb = nc.dram_tensor("bcomb", [1, 1536], F32R, kind="ExternalInput")
    bhn = nc.dram_tensor("bhn", [1, 512], BF16, kind="ExternalInput")
    m2d = nc.dram_tensor("m2", [64, t_steps], F32, kind="ExternalInput")
    h0d = nc.dram_tensor("h0", [32, 512], F32, kind="ExternalInput")
    ysd = nc.dram_tensor("ys", [t_steps, 32, 512], F32, kind="ExternalOutput")

    with tile.TileContext(nc) as tc:
        with (
            tc.tile_pool(name="const", bufs=1) as cpool,
            tc.tile_pool(name="wpool", bufs=1) as wpool,
            tc.tile_pool(name="gx", bufs=2 * GX_PER_BLK) as gxpool,
            tc.tile_pool(name="xstage", bufs=3) as xpool,
            tc.tile_pool(name="ev", bufs=2) as evpool,
            tc.tile_pool(name="ht", bufs=2) as htpool,
            tc.tile_pool(name="psA", bufs=2, space="PSUM") as psA,
            tc.tile_pool(name="psB", bufs=2, space="PSUM") as psB,
            tc.tile_pool(name="psT", bufs=2, space="PSUM") as psT,
            tc.tile_pool(name="psG", bufs=2, space="PSUM") as psG,
        ):
            # ---- constants / weights ----
            import ml_dtypes
            id128_d = nc.inline_tensor(
                np.eye(128).astype(ml_dtypes.bfloat16), name="id128c")
            id64_d = nc.inline_tensor(np.eye(64, dtype=np.float32), name="id64c")
            ones_d = nc.inline_tensor(np.ones((1, 128), np.float32), name="onesc")
            ident128b = cpool.tile([128, 128], BF16, tag="id128")
            nc.sync.dma_start(ident128b[:, :], id128_d.ap())
            ident64 = cpool.tile([64, 64], F32, tag="id64")
            nc.sync.dma_start(ident64[:, :], id64_d.ap())
            ones1 = cpool.tile([1, 128], F32R, tag="ones")
            nc.sync.dma_start(ones1[:, :], ones_d.ap().bitcast(F32R))
            onesb_d = nc.inline_tensor(
                np.ones((1, 128)).astype(ml_dtypes.bfloat16), name="onesbc")
            ones1b = cpool.tile([1, 128], BF16, tag="onesb")
            nc.sync.dma_start(ones1b[:, :], onesb_d.ap())

            wih = [wpool.tile([128, 1536], F32R, tag=f"wih{k}", name=f"wih{k}") for k in range(4)]
            whh = [wpool.tile([128, 1536], BF16, tag=f"whh{k}", name=f"whh{k}") for k in range(4)]
            for k in range(4):
                nc.sync.dma_start(wih[k][:, :], wih_t[128 * k:128 * (k + 1), :])
                nc.sync.dma_start(whh[k][:, :], whh_t[128 * k:128 * (k + 1), :])
            bcomb_sb = cpool.tile([1, 1536], F32R, tag="bcomb")
            nc.sync.dma_start(bcomb_sb[:, :], bcomb[:, :])
            bhn_sb = cpool.tile([1, 512], BF16, tag="bhn")
            nc.sync.dma_start(bhn_sb[:, :], bhn[:, :])
            m2sb = cpool.tile([64, t_steps], F32, tag="m2")
            nc.sync.dma_start(m2sb[:, :], m2d[:, :])

            # ---- phase A emitters ----
            gx_tiles = {}

            def phase_a_dma(g):
                xs = xpool.tile([128, 512], F32R, tag="xstage", name="xs")
                nc.sync.dma_start(xs[:, :], xt[g, :, :])
                gxt = gxpool.tile([128, 1536], BF16, tag="gx", name="gxt")
                gx_tiles[g] = (gxt, xs)
                return gxt, xs

            def phase_a_chunk_mm(g, nchunk):
                gxt, xs = gx_tiles[g]
                gp = psG.tile([128, 512], F32, tag="gxp", name="gp")
                nc.tensor.matmul(
                    gp[:, :], r32(ones1[:, :]),
                    r32(bcomb_sb[:, 512 * nchunk:512 * (nchunk + 1)]),
                    start=True, stop=False)
                for k in range(4):
                    nc.tensor.matmul(
                        gp[:, :], r32(xs[:, 128 * k:128 * (k + 1)]),
                        r32(wih[k][:, 512 * nchunk:512 * (nchunk + 1)]),
                        start=False, stop=(k == 3))
                return gp

            def phase_a_chunk_copy(g, nchunk, gp):
                gxt, _ = gx_tiles[g]
                nc.vector.tensor_copy(
                    gxt[:, 512 * nchunk:512 * (nchunk + 1)], gp[:, :])

            def phase_a_chunk(g, nchunk):
                gp = phase_a_chunk_mm(g, nchunk)
                phase_a_chunk_copy(g, nchunk, gp)

            # ---- prologue: h0 -> half-major layout, initial masked transpose ----
            h_cur = evpool.tile([64, 256], F32, tag="h")
            nc.sync.dma_start(
                h_cur[:, :], h0d.ap().rearrange("e (h j) -> h e j", h=2))

            def emit_diag(t_next):
                diagm = evpool.tile([64, 64], F32, tag="diagm", name="diagm")
                nc.scalar.activation(diagm[:, :], ident64[:, :], AF.Copy,
                                     scale=m2sb[:, t_next:t_next + 1])
                return diagm

            def emit_transpose0(h_src, t_next):
                """Prologue: hT = transpose(diag(m[t_next]) applied to h_src)."""
                diagm = emit_diag(t_next)
                tp = psT.tile([128, 128], F32, tag="htp", name="tp")
                ht = htpool.tile([128, 128], BF16, tag="ht", name="ht")
                for c in range(2):
                    nc.tensor.matmul(
                        tp[:, 64 * c:64 * (c + 1)],
                        h_src[:, 128 * c:128 * (c + 1)].bitcast(F32),
                        diagm[:, :], start=True, stop=True,
                        skip_group_check=True)
                    nc.scalar.activation(
                        ht[:, 64 * c:64 * (c + 1)],
                        tp[:, 64 * c:64 * (c + 1)], AF.Copy)
                return ht

            ht_cur = emit_transpose0(h_cur, 0)

            # phase A for block 0 (not interleaved)
            for g in range(GX_PER_BLK):
                phase_a_dma(g)
                for nchunk in range(3):
                    phase_a_chunk(g, nchunk)

            # ---- main loop ----
            for blk in range(nblk):
                for i in range(TB):
                    t = blk * TB + i
                    g = t // 4
                    b = 32 * (t % 4)
                    gxt = gx_tiles[g][0]

                    pa = psA.tile([128, 256], F32, tag="pa")
                    pb = psB.tile([128, 256], F32, tag="pb")

                    # W0a (bank B): bias(hn) + xn selector
                    nc.tensor.matmul(pb[0:32, :], ones1b[:, 0:32],
                                     bhn_sb[:, 0:256],
                                     start=True, stop=False,
                                     skip_group_check=True,
                                     tile_position=(0, 0))
                    nc.tensor.matmul(pb[32:64, :], ones1b[:, 0:32],
                                     bhn_sb[:, 256:512],
                                     start=True, stop=False,
                                     skip_group_check=True,
                                     tile_position=(0, 32))
                    nc.tensor.matmul(pb[64:96, :], ident128b[:, b:b + 32],
                                     gxt[:, 1024:1280],
                                     start=True, stop=True,
                                     skip_group_check=True,
                                     tile_position=(0, 64))
                    nc.tensor.matmul(pb[96:128, :], ident128b[:, b:b + 32],
                                     gxt[:, 1280:1536],
                                     start=True, stop=True,
                                     skip_group_check=True,
                                     tile_position=(0, 96))
                    # W0b (bank A): xz + xr selector
                    nc.tensor.matmul(pa[0:32, :], ident128b[:, b:b + 32],
                                     gxt[:, 512:768],
                                     start=True, stop=False,
                                     skip_group_check=True,
                                     tile_position=(0, 0))
                    nc.tensor.matmul(pa[32:64, :], ident128b[:, b:b + 32],
                                     gxt[:, 768:1024],
                                     start=True, stop=False,
                                     skip_group_check=True,
                                     tile_position=(0, 32))
                    nc.tensor.matmul(pa[64:96, :], ident128b[:, b:b + 32],
                                     gxt[:, 0:256],
                                     start=True, stop=False,
                                     skip_group_check=True,
                                     tile_position=(0, 64))
                    nc.tensor.matmul(pa[96:128, :], ident128b[:, b:b + 32],
                                     gxt[:, 256:512],
                                     start=True, stop=False,
                                     skip_group_check=True,
                                     tile_position=(0, 96))
                    # W1-4: hn (bank B groups 0,1) + r (bank A groups 2,3)
                    for k in (0, 2, 1, 3):
                        lh = ht_cur[:, CHUNK_COLS[k]:CHUNK_COLS[k] + 32]
                        nc.tensor.matmul(pb[0:32, :], lh,
                                         whh[k][:, 1024:1280],
                                         start=False, stop=(k == 3),
                                         skip_group_check=True,
                                         tile_position=(0, 0))
                        nc.tensor.matmul(pb[32:64, :], lh,
                                         whh[k][:, 1280:1536],
                                         start=False, stop=(k == 3),
                                         skip_group_check=True,
                                         tile_position=(0, 32))
                        nc.tensor.matmul(pa[64:96, :], lh,
                                         whh[k][:, 0:256],
                                         start=False, stop=(k == 3),
                                         skip_group_check=True,
                                         tile_position=(0, 64))
                        nc.tensor.matmul(pa[96:128, :], lh,
                                         whh[k][:, 256:512],
                                         start=False, stop=(k == 3),
                                         skip_group_check=True,
                                         tile_position=(0, 96))
                    # W5-8: z (bank A groups 0,1)
                    for k in (0, 2, 1, 3):
                        lh = ht_cur[:, CHUNK_COLS[k]:CHUNK_COLS[k] + 32]
                        nc.tensor.matmul(pa[0:32, :], lh,
                                         whh[k][:, 512:768],
                                         start=False, stop=(k == 3),
                                         skip_group_check=True,
                                         tile_position=(0, 0))
                        nc.tensor.matmul(pa[32:64, :], lh,
                                         whh[k][:, 768:1024],
                                         start=False, stop=(k == 3),
                                         skip_group_check=True,
                                         tile_position=(0, 32))

                    # interleaved phase A quantum for next block (MM part)
                    pa_copy = None
                    if blk + 1 < nblk:
                        gq = (blk + 1) * GX_PER_BLK + i // 4
                        quantum = i % 4
                        if quantum == 0:
                            phase_a_dma(gq)
                        else:
                            gp_q = phase_a_chunk_mm(gq, quantum - 1)
                            pa_copy = (gq, quantum - 1, gp_q)

                    # ---- elementwise tail ----
                    r_sb = evpool.tile([64, 256], F32, tag="r")
                    nc.scalar.activation(r_sb[:, :], pa[64:128, :], AF.Sigmoid)
                    z_sb = evpool.tile([64, 256], F32, tag="z")
                    nc.scalar.activation(z_sb[:, :], pa[0:64, :], AF.Sigmoid)
                    zc_sb = evpool.tile([64, 256], F32, tag="zc")
                    nc.scalar.activation(zc_sb[:, :], pa[0:64, :], AF.Sigmoid,
                                         scale=-1.0)
                    p_sb = evpool.tile([64, 256], F32, tag="p")
                    nc.vector.tensor_tensor(p_sb[:, :], r_sb[:, :],
                                            pb[0:64, :], OP.mult)
                    q_sb = evpool.tile([64, 256], F32, tag="q")
                    nc.vector.tensor_tensor(q_sb[:, :], p_sb[:, :],
                                            pb[64:128, :], OP.add)
                    # off-chain: w = z * m_t * h_prev
                    w_sb = evpool.tile([64, 256], F32, tag="w")
                    nc.vector.scalar_tensor_tensor(
                        w_sb[:, :], z_sb[:, :], m2sb[:, t:t + 1], h_cur[:, :],
                        OP.mult, OP.mult)
                    n_sb = evpool.tile([64, 256], F32, tag="n")
                    nc.scalar.activation(n_sb[:, :], q_sb[:, :], AF.Tanh)
                    last = t + 1 >= t_steps
                    if not last:
                        diagm = emit_diag(t + 1)
                        tp = psT.tile([128, 256], F32, tag="htp", name="tp")
                        ht = htpool.tile([128, 128], BF16, tag="ht", name="ht")
                        wt_sb = htpool.tile([128, 128], F32, tag="wt", name="wt")
                        for c in range(2):
                            nc.tensor.matmul(
                                tp[:, 64 * c:64 * (c + 1)],
                                w_sb[:, 128 * c:128 * (c + 1)].bitcast(F32),
                                diagm[:, :], start=True, stop=True,
                                skip_group_check=True)
                            nc.scalar.activation(
                                wt_sb[:, 64 * c:64 * (c + 1)],
                                tp[:, 64 * c:64 * (c + 1)], AF.Copy)
                    u_sb = evpool.tile([64, 256], F32, tag="u")
                    nc.vector.tensor_tensor(u_sb[:, :], n_sb[:, :],
                                            zc_sb[:, :], OP.mult)
                    if not last:
                        for c in range(2):
                            nc.tensor.matmul(
                                tp[:, 128 + 64 * c:128 + 64 * (c + 1)],
                                u_sb[:, 128 * c:128 * (c + 1)].bitcast(F32),
                                diagm[:, :], start=True, stop=True,
                                skip_group_check=True)
                        nc.vector.tensor_tensor(
                            ht[:, :], tp[:, 128:256], wt_sb[:, :], OP.add)
                        ht_cur = ht
                    h_new = evpool.tile([64, 256], F32, tag="h")
                    nc.vector.tensor_tensor(h_new[:, :], u_sb[:, :],
                                            w_sb[:, :], OP.add)
                    nc.sync.dma_start(
                        ysd[t].rearrange("e (h j) -> h e j", h=2),
                        h_new[:, :])
                    if pa_copy is not None:
                        phase_a_chunk_copy(*pa_copy)
                    h_cur = h_new
    nc.compile()
    return nc


def _prep_core_inputs(x3, m3, h0, t_steps):
    """x3: (T, 32, 512), m3: (T, 32), h0: (32, 512) -> in_map dict."""
    # xt[g, dd, (k, tt, e)] = x3[4g + tt, e, 128k + dd]
    x4 = x3.reshape(t_steps // 4, 4, NLOC, 4, 128)        # g, tt, e, k, dd
    xt = np.ascontiguousarray(x4.transpose(0, 4, 3, 1, 2)  # g, dd, k, tt, e
                              ).reshape(t_steps // 4, 128, 512)
    m2 = np.ascontiguousarray(
        np.concatenate([m3.T, m3.T], axis=0))              # (64, T)
    return {"xt": xt, "m2": m2, "h0": np.ascontiguousarray(h0)}


def kernel(x, hxs, masks, w_ih, w_hh, b_ih, b_hh, _nc_cache={}):
    t_steps = T
    x = np.asarray(x, np.float32)
    hxs = np.asarray(hxs, np.float32)
    masks = np.asarray(masks, np.float32)
    w_ih = np.asarray(w_ih, np.float32)
    w_hh = np.asarray(w_hh, np.float32)
    b_ih = np.asarray(b_ih, np.float32)
    b_hh = np.asarray(b_hh, np.float32)

    import ml_dtypes
    wih_t = np.ascontiguousarray(w_ih.T)                   # (512, 1536)
    whh_t = np.ascontiguousarray(w_hh.T).astype(ml_dtypes.bfloat16)
    bc = b_ih.copy()
    bc[:1024] += b_hh[:1024]
    bcomb = bc[None, :]                                    # (1, 1536)
    bhn = b_hh[None, 1024:1536].astype(ml_dtypes.bfloat16)  # (1, 512)

    x3f = x.reshape(t_steps, N, D)
    m3f = masks.reshape(t_steps, N)
    h0f = hxs[:, 0, :]                                     # (N, H)

    shared = {"wih_t": wih_t, "whh_t": whh_t, "bcomb": bcomb, "bhn": bhn}
    in_maps = []
    for c in range(NCORE):
        sl = slice(NLOC * c, NLOC * (c + 1))
        im = _prep_core_inputs(
            np.ascontiguousarray(x3f[:, sl]), m3f[:, sl], h0f[sl], t_steps)
        im.update({k: v.copy() for k, v in shared.items()})
        in_maps.append(im)

    if "nc" not in _nc_cache:
        _nc_cache["nc"] = build_nc(t_steps)
    nc = _nc_cache["nc"]
    res = bass_utils.run_bass_kernel_spmd(nc, in_maps, core_ids=list(range(NCORE)))
    ys = np.concatenate(
        [r["ys"] for r in res.results], axis=1)            # (T, 256, 512)
    ys_flat = np.ascontiguousarray(ys).reshape(t_steps * N, H)
    h_last = np.ascontiguousarray(ys[-1])[None]            # (1, N, H)
    return ys_flat, h_last
